# revision 20
# baseline (speedup 1.0000x reference)
"""Trainium2 Bass kernel for a multi-head GQA attention block (dense transformer).

Problem: B=2, S=2048, D=4096, H=32 query heads, HKV=8 KV heads, HD=128.
    q = x @ wq.T + bq; k,v likewise; GQA causal attention; out @ wo.T + bo.

Sharding: tensor-parallel over heads across 8 cores. Core r owns query
heads [4r, 4r+4) and KV head r (GQA groups align with the split). Each
core projects q/k/v for its heads from the full x, runs attention, then
all-gathers the per-core attention outputs (o) on-device and computes its
512-wide slice of the output projection. The host concatenates the 8
output slices.

All matmuls run in float32r (full-rate fp32 PE mode, ~1.5e-4 rms rounding)
with fp32 PSUM accumulation.

Layouts (everything "transposed", tokens on the free axis):
  xT   [D, T]    T = B*S = 4096 tokens
  qT   [hd, t] per head (spilled to DRAM);  kT [hd, t];
  v as [tk, hd] tiles (PE-transposed)
  scoresT[tk, tq] = kT_tile.T @ qT  -> softmax over tk (partition axis):
     exp on ACT, row-sums via ones-vector matmuls, normalize at the end.
  oT   [e_local=512, *] -> chunked AllGather (4 token chunks, overlapped
  with attention) -> o_proj -> yT [512, T]
"""

import math

import numpy as np

import concourse.bass as bass
import concourse.tile as tile
from concourse import bacc, mybir
from concourse.bass_utils import run_bass_kernel_spmd

# Problem constants (hardcoded per contest contract)
B, S, D = 2, 2048, 4096
H, HKV, HD = 32, 8, 128
T = B * S                      # 4096 tokens
NCORES = 8
HL = H // NCORES               # 4 query heads per core
EQ = HL * HD                   # 512 q features per core
EKV = HD                       # 128 kv features per core
KT = D // 128                  # 32 contraction tiles
PCH = 256                      # projection token-chunk
TQ = 512                       # attention query chunk
TCH = 512                      # all-gather token chunk
SCALE = 1.0 / math.sqrt(HD)

F32 = mybir.dt.float32
F32R = mybir.dt.float32r
BF16 = mybir.dt.bfloat16

CDT = F32R                     # compute dtype fed to the PE
NEG = -30000.0                 # additive causal mask value (exp -> 0)

_cache = {}

DG = 8                         # k-tiles per DMA transfer (~1MB batches)


def _load_w(nc, dst, src3, width):
    for i, k0 in enumerate(range(0, KT, DG)):
        eng = nc.sync if i % 2 == 0 else nc.scalar
        eng.dma_start(dst[:, k0:k0 + DG, :], src3[:, k0:k0 + DG, :])


def _load_x(nc, dst, src3, ts, w):
    for i, k0 in enumerate(range(0, KT, DG)):
        eng = nc.sync if i % 2 == 0 else nc.scalar
        eng.dma_start(dst[:, k0:k0 + DG, :], src3[:, k0:k0 + DG, ts:ts + w])


def build_bass(single_core: bool = False):
    nc = bacc.Bacc("TRN2", target_bir_lowering=False, debug=False,
                   num_swdge_queues=4,
                   num_devices=1 if single_core else NCORES)

    dram = {}
    def din(name, shape, dt=CDT):
        dram[name] = nc.dram_tensor(name, shape, dt, kind="ExternalInput").ap()
        return dram[name]

    xT = din("xT", [D, T])
    wqT = din("wqT", [D, EQ])
    wkT = din("wkT", [D, EKV])
    wvT = din("wvT", [D, EKV])
    woT = din("woT", [D, EQ])
    bq = din("bq", [128, HL], F32)
    bk = din("bk", [128, 1], F32)
    bv = din("bv", [128, 1], F32)
    bo = din("bo", [128, HL], F32)
    masks = din("masks", [128, 4, TQ], BF16)   # additive {0, NEG}, diag offsets
    ones = din("ones", [128, 128])
    onesf = din("onesf", [128, 128], F32)
    ident = din("ident", [128, 128])
    yT = nc.dram_tensor("yT", [EQ, T], F32, kind="ExternalOutput").ap()

    xT3 = xT.rearrange("(o p) t -> p o t", p=128)

    with tile.TileContext(nc) as tc:
        with (
            tc.tile_pool(name="const", bufs=1) as constp,
            tc.tile_pool(name="dram", bufs=1, space="DRAM") as dramp,
        ):
            masks_sb = constp.tile([128, 4, TQ], BF16, tag="masks")
            nc.sync.dma_start(masks_sb[:], masks[:, :, :])
            ones_sb = constp.tile([128, 128], CDT, tag="ones")
            nc.sync.dma_start(ones_sb[:], ones[:, :])
            onesf_sb = constp.tile([128, 128], F32, tag="onesf")
            nc.sync.dma_start(onesf_sb[:], onesf[:, :])
            ident_sb = constp.tile([128, 128], CDT, tag="ident")
            nc.sync.dma_start(ident_sb[:], ident[:, :])
            bq_sb = constp.tile([128, HL], F32, tag="bq")
            nc.sync.dma_start(bq_sb[:], bq[:, :])
            bk_sb = constp.tile([128, 1], F32, tag="bk")
            nc.sync.dma_start(bk_sb[:], bk[:, :])
            bv_sb = constp.tile([128, 1], F32, tag="bv")
            nc.sync.dma_start(bv_sb[:], bv[:, :])
            bo_sb = constp.tile([128, HL], F32, tag="bo")
            nc.sync.dma_start(bo_sb[:], bo[:, :])

            NTC = T // TCH
            o_in_c = [dramp.tile([EQ, TCH], CDT, name=f"o_in{i}") for i in range(NTC)]
            o_all_c = [dramp.tile([D, TCH], CDT, name=f"o_all{i}") for i in range(NTC)]
            q_dram = [dramp.tile([EQ, S], CDT, name=f"q_dram{b}") for b in range(B)]

            kvt = {}
            with tc.tile_pool(name="kvp", bufs=1) as kvp:
                for b in range(B):
                    kvt[b] = (kvp.tile([128, S], CDT, tag=f"k{b}", name=f"k{b}"),
                              kvp.tile([128, S // 128, 128], CDT, tag=f"v{b}", name=f"v{b}"))

                # ---- projections (both batches) with all weights resident ----
                with tc.tile_pool(name="wall", bufs=1) as wallp:
                    wq_sb = wallp.tile([128, KT, EQ], CDT, tag="wq")
                    wk_sb = wallp.tile([128, KT, EKV], CDT, tag="wk")
                    wv_sb = wallp.tile([128, KT, EKV], CDT, tag="wv")
                    _load_w(nc, wk_sb, wkT.rearrange("(o p) e -> p o e", p=128), EKV)
                    _load_w(nc, wv_sb, wvT.rearrange("(o p) e -> p o e", p=128), EKV)
                    # wq is loaded after the first x chunk is queued (startup)
                    wq_loader = [lambda: _load_w(
                        nc, wq_sb, wqT.rearrange("(o p) e -> p o e", p=128), EQ)]
                    for b in range(B):
                        _proj_batch(nc, tc, b, xT3, wq_sb, wk_sb, wv_sb,
                                    bq_sb, bk_sb, bv_sb, ident_sb,
                                    kvt[b][0], kvt[b][1], q_dram[b], wq_loader)

                # ---- attention + chunked all-gather + o_proj ----
                with (
                    tc.tile_pool(name="wo", bufs=1) as wop,
                    tc.tile_pool(name="oc", bufs=2) as ocp,
                    tc.tile_pool(name="ys", bufs=2) as ysp,
                    tc.tile_pool(name="att_q", bufs=2) as qcp,
                    tc.tile_pool(name="att_e", bufs=2) as expp,
                    tc.tile_pool(name="att_r", bufs=2) as rpp,
                    tc.tile_pool(name="att_o", bufs=2) as osp,
                    tc.tile_pool(name="scps", bufs=2, space="PSUM") as scpsp,
                    tc.tile_pool(name="oaps", bufs=1, space="PSUM") as oapsp,
                    tc.tile_pool(name="oaps2", bufs=1, space="PSUM") as oapsp2,
                    tc.tile_pool(name="smps", bufs=1, space="PSUM") as smpsp,
                    tc.tile_pool(name="smps2", bufs=1, space="PSUM") as smpsp2,
                ):
                    wo_sb = wop.tile([128, KT, EQ], CDT, tag="wo")
                    wo_loader = [lambda: _load_w(
                        nc, wo_sb, woT.rearrange("(o p) e -> p o e", p=128), EQ)]
                    pools = (qcp, expp, rpp, osp, scpsp, oapsp, oapsp2, smpsp, smpsp2)
                    for b in range(B):
                        for c in range(S // TQ):
                            _att_chunk(nc, tc, b, c, kvt[b][0], kvt[b][1], q_dram[b],
                                       masks_sb, ones_sb, onesf_sb, o_in_c, pools)
                            if wo_loader:
                                wo_loader.pop()()
                            i = b * (S // TQ) + c
                            if single_core:
                                nc.sync.dma_start(o_all_c[i][0:EQ, :], o_in_c[i][:, :])
                            else:
                                nc.gpsimd.collective_compute(
                                    "AllGather", mybir.AluOpType.bypass,
                                    replica_groups=[list(range(NCORES))],
                                    ins=[o_in_c[i].opt()],
                                    outs=[o_all_c[i].opt()])
                            if i > 0:
                                _oproj_chunk(nc, i - 1, o_all_c[i - 1], wo_sb, bo_sb,
                                             yT, ocp, ysp, (smpsp, smpsp2))
                    last = T // TCH - 1
                    _oproj_chunk(nc, last, o_all_c[last], wo_sb, bo_sb, yT,
                                 ocp, ysp, (smpsp, smpsp2))

    nc.compile()
    return nc


def _proj_batch(nc, tc, b, xT3, wq_sb, wk_sb, wv_sb, bq_sb, bk_sb, bv_sb,
                ident_sb, k_sb, v_sb, q_dram, wq_loader):
    CB = S // PCH
    t0 = b * S
    with (
        tc.tile_pool(name="x1", bufs=2) as xp,
        tc.tile_pool(name="vt", bufs=2) as vtp,
        tc.tile_pool(name="qst", bufs=3) as qstp,
        tc.tile_pool(name="pps", bufs=3, space="PSUM") as ppsp,
        tc.tile_pool(name="tps", bufs=2, space="PSUM") as tpsp,
    ):
        for c in range(CB):
            ts = t0 + c * PCH
            x_sb = xp.tile([128, KT, PCH], CDT, tag="x")
            _load_x(nc, x_sb, xT3, ts, PCH)
            if wq_loader:
                wq_loader.pop()()
            ps = ppsp.tile([128, PCH], F32, tag="pps")
            for k in range(KT):
                nc.tensor.matmul(ps[:], wk_sb[:, k, :], x_sb[:, k, :],
                                 start=(k == 0), stop=(k == KT - 1))
            nc.vector.tensor_scalar_add(k_sb[:, c * PCH:(c + 1) * PCH], ps[:], bk_sb[:, 0:1])
            ps = ppsp.tile([128, PCH], F32, tag="pps")
            for k in range(KT):
                nc.tensor.matmul(ps[:], wv_sb[:, k, :], x_sb[:, k, :],
                                 start=(k == 0), stop=(k == KT - 1))
            vt_sb = vtp.tile([128, PCH], CDT, tag="vt")
            nc.vector.tensor_scalar_add(vt_sb[:], ps[:], bv_sb[:, 0:1])
            # transpose [hd, t] tiles into v_sb [t, hd] tiles
            for s4 in range(PCH // 128):
                pt = tpsp.tile([128, 128], CDT, tag="tps")
                nc.tensor.transpose(pt[:], vt_sb[:, s4 * 128:(s4 + 1) * 128], ident_sb[:])
                nc.vector.tensor_copy(v_sb[:, c * (PCH // 128) + s4, :], pt[:])
            for e in range(HL):
                ps = ppsp.tile([128, PCH], F32, tag="pps")
                for k in range(KT):
                    nc.tensor.matmul(ps[:], wq_sb[:, k, e * 128:(e + 1) * 128],
                                     x_sb[:, k, :], start=(k == 0), stop=(k == KT - 1))
                qst = qstp.tile([128, PCH], CDT, tag="qst")
                nc.vector.tensor_scalar_add(qst[:], ps[:], bq_sb[:, e:e + 1])
                nc.gpsimd.dma_start(
                    q_dram[e * 128:(e + 1) * 128, ts - t0:ts - t0 + PCH], qst[:])


def _att_chunk(nc, tc, b, c, k_sb, v_sb, q_dram, masks_sb, ones_sb, onesf_sb,
               o_in_c, pools):
    (qcp, expp, rpp, osp, scpsp, oapsp, oapsp2, smpsp, smpsp2) = pools
    t0 = b * S
    tqs = c * TQ                   # within-batch query offset
    n_tk = (c + 1) * (TQ // 128)   # causal tk tiles
    q_dram3 = q_dram.rearrange("(h p) t -> p h t", p=128)

    qc = qcp.tile([128, HL, TQ], CDT, tag="qc", name="qc")
    nc.sync.dma_start(qc[:], q_dram3[:, :, tqs:tqs + TQ])
    pending = []

    def epilogue(out_ps, sums_ps, g):
        gt = t0 + tqs                    # global token offset
        i = gt // TCH
        off = gt % TCH
        for hh in range(2):
            recip_sb = rpp.tile([1, TQ], F32, tag="recip", name="recip")
            nc.vector.reciprocal_approx_fast(recip_sb[:], sums_ps[hh][0:1, :])
            recip_r = rpp.tile([1, TQ], CDT, tag="recipr", name="recip_r")
            nc.vector.tensor_copy(recip_r[:], recip_sb[:])
            rb = scpsp.tile([128, 2, TQ], F32, tag="sc", name="rb")
            nc.tensor.matmul(rb[:, 0, :], ones_sb[0:1, :],
                             recip_r[0:1, :], start=True, stop=True)
            rb_sb = rpp.tile([128, TQ], F32, tag="rb", name="rb_sb")
            nc.scalar.copy(rb_sb[:], rb[:, 0, :])
            o_sb = osp.tile([128, TQ], CDT, tag="o", name="o")
            nc.vector.tensor_tensor(o_sb[:], out_ps[hh][:], rb_sb[:],
                                    mybir.AluOpType.mult)
            h = 2 * g + hh
            nc.gpsimd.dma_start(o_in_c[i][h * 128:(h + 1) * 128, off:off + TQ], o_sb[:])

    for g in range(HL // 2):       # head pairs
        out_ps = [oapsp.tile([128, TQ], F32, tag="oa0", name="oa0"),
                  oapsp2.tile([128, TQ], F32, tag="oa1", name="oa1")]
        sums_ps = [smpsp.tile([128, TQ], F32, tag="sums0", name="sums0"),
                   smpsp2.tile([128, TQ], F32, tag="sums1", name="sums1")]
        for tkt in range(n_tk):
            sc = scpsp.tile([128, 2, TQ], F32, tag="sc", name="sc")
            for hh in range(2):
                h = 2 * g + hh
                nc.tensor.matmul(
                    sc[:, hh, :],
                    k_sb[:, tkt * 128:(tkt + 1) * 128],
                    qc[:, h, :],
                    start=True, stop=True)
            diag = tkt - (n_tk - 4)
            if diag >= 0:
                nc.vector.tensor_tensor(
                    sc[:], sc[:],
                    masks_sb[:, diag:diag + 1, :].to_broadcast([128, 2, TQ]),
                    mybir.AluOpType.add)
            exp_sb = expp.tile([128, 2, TQ], CDT, tag="exp", name="exp")
            nc.scalar.activation(exp_sb[:], sc[:],
                                 mybir.ActivationFunctionType.Exp, scale=SCALE)
            for hh in range(2):
                nc.tensor.matmul(out_ps[hh][:], v_sb[:, tkt, :], exp_sb[:, hh, :],
                                 start=(tkt == 0), stop=(tkt == n_tk - 1),
                                 skip_group_check=True)
            for hh in range(2):
                nc.tensor.matmul(sums_ps[hh][0:1, :], ones_sb[:, 0:1],
                                 exp_sb[:, hh, :],
                                 start=(tkt == 0), stop=(tkt == n_tk - 1),
                                 skip_group_check=True)
            if tkt == 0 and pending:
                pending.pop()()
        pending.append(lambda o=out_ps, s=sums_ps, gg=g: epilogue(o, s, gg))
    while pending:
        pending.pop()()


def _oproj_chunk(nc, i, o_all_i, wo_sb, bo_sb, yT, ocp, ysp, opsps):
    """yT[:, i*TCH : (i+1)*TCH] = wo_local @ o_all_i (+ bo slice)."""
    o3 = o_all_i.rearrange("(o p) t -> p o t", p=128)
    for cc in range(TCH // PCH):
        oc_sb = ocp.tile([128, KT, PCH], CDT, tag="oc", name="oc")
        _load_x(nc, oc_sb, o3, cc * PCH, PCH)
        for e in range(HL):
            ps = opsps[e % 2].tile([128, TQ], F32, tag=f"sums{e % 2}",
                                   name=f"sums{e % 2}")[:, :PCH]
            for k in range(KT):
                nc.tensor.matmul(ps[:], wo_sb[:, k, e * 128:(e + 1) * 128],
                                 oc_sb[:, k, :], start=(k == 0), stop=(k == KT - 1))
            y_sb = ysp.tile([128, PCH], F32, tag="ys", name="ys")
            nc.vector.tensor_scalar_add(y_sb[:], ps[:], bo_sb[:, e:e + 1])
            t_lo = i * TCH + cc * PCH
            nc.gpsimd.dma_start(yT[e * 128:(e + 1) * 128, t_lo:t_lo + PCH], y_sb[:])


def _host_inputs(core, inputs):
    """Per-core numpy input map (all fp32; f32r is an interpretation)."""
    x = np.asarray(inputs["x"], np.float32)
    xT = np.ascontiguousarray(x.reshape(T, D).T)

    r0, r1 = core * EQ, (core + 1) * EQ
    kv0, kv1 = core * EKV, (core + 1) * EKV
    wqT = np.ascontiguousarray(np.asarray(inputs["wq"], np.float32)[r0:r1].T)
    wkT = np.ascontiguousarray(np.asarray(inputs["wk"], np.float32)[kv0:kv1].T)
    wvT = np.ascontiguousarray(np.asarray(inputs["wv"], np.float32)[kv0:kv1].T)
    woT = np.ascontiguousarray(np.asarray(inputs["wo"], np.float32)[r0:r1].T)
    bq = np.ascontiguousarray(np.asarray(inputs["bq"], np.float32)[r0:r1].reshape(HL, 128).T)
    bk = np.ascontiguousarray(np.asarray(inputs["bk"], np.float32)[kv0:kv1].reshape(1, 128).T)
    bv = np.ascontiguousarray(np.asarray(inputs["bv"], np.float32)[kv0:kv1].reshape(1, 128).T)
    bo = np.ascontiguousarray(np.asarray(inputs["bo"], np.float32)[r0:r1].reshape(HL, 128).T)

    i = np.arange(128)[:, None]
    j = np.arange(TQ)[None, :]
    masks = np.stack([(j < i + 128 * o) * np.float32(NEG) for o in range(4)], axis=1)
    import ml_dtypes
    masks = np.ascontiguousarray(masks.astype(ml_dtypes.bfloat16))

    return {
        "xT": xT, "wqT": wqT, "wkT": wkT, "wvT": wvT, "woT": woT,
        "bq": bq, "bk": bk, "bv": bv, "bo": bo,
        "masks": masks,
        "ones": np.ones((128, 128), np.float32),
        "onesf": np.ones((128, 128), np.float32),
        "ident": np.eye(128, dtype=np.float32),
    }


def kernel(**inputs) -> np.ndarray:
    if "nc" not in _cache:
        _cache["nc"] = build_bass()
    nc = _cache["nc"]
    in_maps = [_host_inputs(r, inputs) for r in range(NCORES)]
    res = run_bass_kernel_spmd(nc, in_maps, list(range(NCORES)))
    yT = np.concatenate([res.results[r]["yT"] for r in range(NCORES)], axis=0)
    return np.ascontiguousarray(yT.T).reshape(B, S, D).astype(np.float32)


if __name__ == "__main__":
    import sys
    if len(sys.argv) > 1 and sys.argv[1] == "buildonly":
        import time
        t0 = time.perf_counter()
        nc = build_bass()
        print(f"build+bacc-compile ok in {time.perf_counter()-t0:.1f}s")
        sys.exit(0)
    import reference
    ins = {k: np.asarray(v) for k, v in reference.setup_inputs().items()}
    out = kernel(**ins)
    print("out", out.shape, out.dtype)


# revision 25
# speedup vs baseline: 74.9807x; 74.9807x over previous
"""Trainium2 Bass kernel for a multi-head GQA attention block (dense transformer).

Problem: B=2, S=2048, D=4096, H=32 query heads, HKV=8 KV heads, HD=128.
    q = x @ wq.T + bq; k,v likewise; GQA causal attention; out @ wo.T + bo.

Sharding: tensor-parallel over heads across 8 cores. Core r owns query
heads [4r, 4r+4) and KV head r (GQA groups align with the split). Each
core projects q/k/v for its heads from the full x, runs attention, then
all-gathers the per-core attention outputs (o) on-device and computes its
512-wide slice of the output projection. The host concatenates the 8
output slices.

All matmuls run in float32r (full-rate fp32 PE mode, ~1.5e-4 rms rounding)
with fp32 PSUM accumulation.

Layouts (everything "transposed", tokens on the free axis):
  xT   [D, T]    T = B*S = 4096 tokens
  qT   [hd, t] per head (spilled to DRAM);  kT [hd, t];
  v as [tk, hd] tiles (PE-transposed)
  scoresT[tk, tq] = kT_tile.T @ qT  -> softmax over tk (partition axis):
     exp on ACT, row-sums via ones-vector matmuls, normalize at the end.
  oT   [e_local=512, *] -> chunked AllGather (4 token chunks, overlapped
  with attention) -> o_proj -> yT [512, T]
"""

import math

import numpy as np

import concourse.bass as bass
import concourse.tile as tile
from concourse import bacc, mybir
from concourse.bass_utils import run_bass_kernel_spmd

# Problem constants (hardcoded per contest contract)
B, S, D = 2, 2048, 4096
H, HKV, HD = 32, 8, 128
T = B * S                      # 4096 tokens
NCORES = 8
HL = H // NCORES               # 4 query heads per core
EQ = HL * HD                   # 512 q features per core
EKV = HD                       # 128 kv features per core
KT = D // 128                  # 32 contraction tiles
PCH = 256                      # projection token-chunk
TQ = 512                       # attention query chunk
TCH = 512                      # all-gather token chunk
SCALE = 1.0 / math.sqrt(HD)

F32 = mybir.dt.float32
F32R = mybir.dt.float32r
BF16 = mybir.dt.bfloat16

CDT = F32R                     # compute dtype fed to the PE
NEG = -30000.0                 # additive causal mask value (exp -> 0)

_cache = {}

DG = 8                         # k-tiles per DMA transfer (~1MB batches)


def _load_w(nc, dst, src3, width):
    for i, k0 in enumerate(range(0, KT, DG)):
        eng = nc.sync if i % 2 == 0 else nc.scalar
        eng.dma_start(dst[:, k0:k0 + DG, :], src3[:, k0:k0 + DG, :])


def _load_x(nc, dst, src3, ts, w):
    for i, k0 in enumerate(range(0, KT, DG)):
        eng = nc.sync if i % 2 == 0 else nc.scalar
        eng.dma_start(dst[:, k0:k0 + DG, :], src3[:, k0:k0 + DG, ts:ts + w])


def build_bass(single_core: bool = False):
    nc = bacc.Bacc("TRN2", target_bir_lowering=False, debug=False,
                   num_swdge_queues=4,
                   num_devices=1 if single_core else NCORES)

    dram = {}
    def din(name, shape, dt=CDT):
        dram[name] = nc.dram_tensor(name, shape, dt, kind="ExternalInput").ap()
        return dram[name]

    xT = din("xT", [D, T])
    wqT = din("wqT", [D, EQ])
    wkT = din("wkT", [D, EKV])
    wvT = din("wvT", [D, EKV])
    woT = din("woT", [D, EQ])
    bq = din("bq", [128, HL], F32)
    bk = din("bk", [128, 1], F32)
    bv = din("bv", [128, 1], F32)
    bo = din("bo", [128, HL], F32)
    masks = din("masks", [128, 4, TQ], BF16)   # additive {0, NEG}, diag offsets
    ones = din("ones", [128, 128])
    onesf = din("onesf", [128, 128], F32)
    ident = din("ident", [128, 128])
    yT = nc.dram_tensor("yT", [EQ, T], F32, kind="ExternalOutput").ap()

    xT3 = xT.rearrange("(o p) t -> p o t", p=128)

    with tile.TileContext(nc) as tc:
        with (
            tc.tile_pool(name="const", bufs=1) as constp,
            tc.tile_pool(name="dram", bufs=1, space="DRAM") as dramp,
        ):
            masks_sb = constp.tile([128, 4, TQ], BF16, tag="masks")
            nc.scalar.dma_start(masks_sb[:], masks[:, :, :])
            ones_sb = constp.tile([128, 128], CDT, tag="ones")
            nc.scalar.dma_start(ones_sb[:], ones[:, :])
            onesf_sb = constp.tile([128, 128], F32, tag="onesf")
            nc.scalar.dma_start(onesf_sb[:], onesf[:, :])
            ident_sb = constp.tile([128, 128], CDT, tag="ident")
            nc.scalar.dma_start(ident_sb[:], ident[:, :])
            bk_sb = constp.tile([128, 1], F32, tag="bk")
            nc.sync.dma_start(bk_sb[:], bk[:, :])
            bq_sb = constp.tile([128, HL], F32, tag="bq")
            nc.scalar.dma_start(bq_sb[:], bq[:, :])
            bv_sb = constp.tile([128, 1], F32, tag="bv")
            nc.scalar.dma_start(bv_sb[:], bv[:, :])
            bo_sb = constp.tile([128, HL], F32, tag="bo")
            nc.scalar.dma_start(bo_sb[:], bo[:, :])

            NTC = T // TCH
            o_in_c = [dramp.tile([EQ, TCH], CDT, name=f"o_in{i}") for i in range(NTC)]
            o_all_c = [dramp.tile([D, TCH], CDT, name=f"o_all{i}") for i in range(NTC)]
            q_dram = [dramp.tile([EQ, S], CDT, name=f"q_dram{b}") for b in range(B)]

            kvt = {}
            with tc.tile_pool(name="kvp", bufs=1) as kvp:
                for b in range(B):
                    kvt[b] = (kvp.tile([128, S], CDT, tag=f"k{b}", name=f"k{b}"),
                              kvp.tile([128, S // 128, 128], CDT, tag=f"v{b}", name=f"v{b}"))

                # ---- projections (both batches) with all weights resident ----
                with tc.tile_pool(name="wall", bufs=1) as wallp:
                    wq_sb = wallp.tile([128, KT, EQ], CDT, tag="wq")
                    wk_sb = wallp.tile([128, KT, EKV], CDT, tag="wk")
                    wv_sb = wallp.tile([128, KT, EKV], CDT, tag="wv")
                    _load_w(nc, wk_sb, wkT.rearrange("(o p) e -> p o e", p=128), EKV)
                    _load_w(nc, wv_sb, wvT.rearrange("(o p) e -> p o e", p=128), EKV)
                    # wq is loaded after the first x chunk is queued (startup)
                    wq_loader = [lambda: _load_w(
                        nc, wq_sb, wqT.rearrange("(o p) e -> p o e", p=128), EQ)]
                    for b in range(B):
                        _proj_batch(nc, tc, b, xT3, wq_sb, wk_sb, wv_sb,
                                    bq_sb, bk_sb, bv_sb, ident_sb,
                                    kvt[b][0], kvt[b][1], q_dram[b], wq_loader)

                # ---- attention + chunked all-gather + o_proj ----
                with (
                    tc.tile_pool(name="wo", bufs=1) as wop,
                    tc.tile_pool(name="oc", bufs=2) as ocp,
                    tc.tile_pool(name="ys", bufs=2) as ysp,
                    tc.tile_pool(name="att_q", bufs=2) as qcp,
                    tc.tile_pool(name="att_e", bufs=3) as expp,
                    tc.tile_pool(name="att_r", bufs=1) as rpp,
                    tc.tile_pool(name="att_o", bufs=2) as osp,
                    tc.tile_pool(name="scps", bufs=2, space="PSUM") as scpsp,
                    tc.tile_pool(name="oaps", bufs=1, space="PSUM") as oapsp,
                    tc.tile_pool(name="oaps2", bufs=1, space="PSUM") as oapsp2,
                    tc.tile_pool(name="smps", bufs=1, space="PSUM") as smpsp,
                    tc.tile_pool(name="smps2", bufs=1, space="PSUM") as smpsp2,
                ):
                    wo_sb = wop.tile([128, KT, EQ], CDT, tag="wo")
                    wo_loader = [lambda: _load_w(
                        nc, wo_sb, woT.rearrange("(o p) e -> p o e", p=128), EQ)]
                    pools = (qcp, expp, rpp, osp, scpsp, oapsp, oapsp2, smpsp, smpsp2)
                    for b in range(B):
                        for c in range(S // TQ):
                            _att_chunk(nc, tc, b, c, kvt[b][0], kvt[b][1], q_dram[b],
                                       masks_sb, ones_sb, onesf_sb, o_in_c, pools)
                            if wo_loader:
                                wo_loader.pop()()
                            i = b * (S // TQ) + c
                            if single_core:
                                nc.sync.dma_start(o_all_c[i][0:EQ, :], o_in_c[i][:, :])
                            else:
                                nc.gpsimd.collective_compute(
                                    "AllGather", mybir.AluOpType.bypass,
                                    replica_groups=[list(range(NCORES))],
                                    ins=[o_in_c[i].opt()],
                                    outs=[o_all_c[i].opt()])
                            if i > 0:
                                _oproj_chunk(nc, i - 1, o_all_c[i - 1], wo_sb, bo_sb,
                                             yT, ocp, ysp, (smpsp, smpsp2))
                    last = T // TCH - 1
                    _oproj_chunk(nc, last, o_all_c[last], wo_sb, bo_sb, yT,
                                 ocp, ysp, (smpsp, smpsp2))

    nc.compile()
    return nc


def _proj_batch(nc, tc, b, xT3, wq_sb, wk_sb, wv_sb, bq_sb, bk_sb, bv_sb,
                ident_sb, k_sb, v_sb, q_dram, wq_loader):
    CB = S // PCH
    t0 = b * S
    with (
        tc.tile_pool(name="x1", bufs=2) as xp,
        tc.tile_pool(name="vt", bufs=2) as vtp,
        tc.tile_pool(name="qst", bufs=3) as qstp,
        tc.tile_pool(name="pps", bufs=3, space="PSUM") as ppsp,
        tc.tile_pool(name="tps", bufs=2, space="PSUM") as tpsp,
    ):
        for c in range(CB):
            ts = t0 + c * PCH
            x_sb = xp.tile([128, KT, PCH], CDT, tag="x")
            _load_x(nc, x_sb, xT3, ts, PCH)
            if wq_loader:
                wq_loader.pop()()
            ps = ppsp.tile([128, PCH], F32, tag="pps")
            for k in range(KT):
                nc.tensor.matmul(ps[:], wk_sb[:, k, :], x_sb[:, k, :],
                                 start=(k == 0), stop=(k == KT - 1))
            nc.vector.tensor_scalar_add(k_sb[:, c * PCH:(c + 1) * PCH], ps[:], bk_sb[:, 0:1])
            ps = ppsp.tile([128, PCH], F32, tag="pps")
            for k in range(KT):
                nc.tensor.matmul(ps[:], wv_sb[:, k, :], x_sb[:, k, :],
                                 start=(k == 0), stop=(k == KT - 1))
            vt_sb = vtp.tile([128, PCH], CDT, tag="vt")
            nc.vector.tensor_scalar_add(vt_sb[:], ps[:], bv_sb[:, 0:1])
            # transpose [hd, t] tiles into v_sb [t, hd] tiles
            for s4 in range(PCH // 128):
                pt = tpsp.tile([128, 128], CDT, tag="tps")
                nc.tensor.transpose(pt[:], vt_sb[:, s4 * 128:(s4 + 1) * 128], ident_sb[:])
                nc.vector.tensor_copy(v_sb[:, c * (PCH // 128) + s4, :], pt[:])
            for e in range(HL):
                ps = ppsp.tile([128, PCH], F32, tag="pps")
                for k in range(KT):
                    nc.tensor.matmul(ps[:], wq_sb[:, k, e * 128:(e + 1) * 128],
                                     x_sb[:, k, :], start=(k == 0), stop=(k == KT - 1))
                qst = qstp.tile([128, PCH], CDT, tag="qst")
                nc.vector.tensor_scalar_add(qst[:], ps[:], bq_sb[:, e:e + 1])
                nc.gpsimd.dma_start(
                    q_dram[e * 128:(e + 1) * 128, ts - t0:ts - t0 + PCH], qst[:])


def _att_chunk(nc, tc, b, c, k_sb, v_sb, q_dram, masks_sb, ones_sb, onesf_sb,
               o_in_c, pools):
    (qcp, expp, rpp, osp, scpsp, oapsp, oapsp2, smpsp, smpsp2) = pools
    t0 = b * S
    tqs = c * TQ                   # within-batch query offset
    n_tk = (c + 1) * (TQ // 128)   # causal tk tiles
    q_dram3 = q_dram.rearrange("(h p) t -> p h t", p=128)

    qc = qcp.tile([128, HL, TQ], CDT, tag="qc", name="qc")
    nc.sync.dma_start(qc[:], q_dram3[:, :, tqs:tqs + TQ])
    pending = []

    def epilogue(out_ps, sums_ps, g):
        gt = t0 + tqs                    # global token offset
        i = gt // TCH
        off = gt % TCH
        for hh in range(2):
            recip_sb = rpp.tile([1, TQ], F32, tag="recip", name="recip")
            nc.vector.reciprocal_approx_fast(recip_sb[:], sums_ps[hh][0:1, :])
            recip_r = rpp.tile([1, TQ], CDT, tag="recipr", name="recip_r")
            nc.vector.tensor_copy(recip_r[:], recip_sb[:])
            rb = scpsp.tile([128, 2, TQ], F32, tag="sc", name="rb")
            nc.tensor.matmul(rb[:, 0, :], ones_sb[0:1, :],
                             recip_r[0:1, :], start=True, stop=True)
            rb_sb = rpp.tile([128, TQ], F32, tag="rb", name="rb_sb")
            nc.scalar.copy(rb_sb[:], rb[:, 0, :])
            o_sb = osp.tile([128, TQ], CDT, tag="o", name="o")
            nc.vector.tensor_tensor(o_sb[:], out_ps[hh][:], rb_sb[:],
                                    mybir.AluOpType.mult)
            h = 2 * g + hh
            nc.gpsimd.dma_start(o_in_c[i][h * 128:(h + 1) * 128, off:off + TQ], o_sb[:])

    for g in range(HL // 2):       # head pairs
        out_ps = [oapsp.tile([128, TQ], F32, tag="oa0", name="oa0"),
                  oapsp2.tile([128, TQ], F32, tag="oa1", name="oa1")]
        sums_ps = [smpsp.tile([128, TQ], F32, tag="sums0", name="sums0"),
                   smpsp2.tile([128, TQ], F32, tag="sums1", name="sums1")]
        def consume(exp_sb, tkt):
            for hh in range(2):
                nc.tensor.matmul(out_ps[hh][:], v_sb[:, tkt, :], exp_sb[:, hh, :],
                                 start=(tkt == 0), stop=(tkt == n_tk - 1),
                                 skip_group_check=True)
            for hh in range(2):
                nc.tensor.matmul(sums_ps[hh][0:1, :], ones_sb[:, 0:1],
                                 exp_sb[:, hh, :],
                                 start=(tkt == 0), stop=(tkt == n_tk - 1),
                                 skip_group_check=True)

        prev = None
        for tkt in range(n_tk):
            sc = scpsp.tile([128, 2, TQ], F32, tag="sc", name="sc")
            for hh in range(2):
                h = 2 * g + hh
                nc.tensor.matmul(
                    sc[:, hh, :],
                    k_sb[:, tkt * 128:(tkt + 1) * 128],
                    qc[:, h, :],
                    start=True, stop=True)
            diag = tkt - (n_tk - 4)
            if diag >= 0:
                nc.vector.tensor_tensor(
                    sc[:], sc[:],
                    masks_sb[:, diag:diag + 1, :].to_broadcast([128, 2, TQ]),
                    mybir.AluOpType.add)
            exp_sb = expp.tile([128, 2, TQ], CDT, tag="exp", name="exp")
            nc.scalar.activation(exp_sb[:], sc[:],
                                 mybir.ActivationFunctionType.Exp, scale=SCALE)
            if prev is not None:
                consume(*prev)
            elif pending:
                pending.pop()()
            prev = (exp_sb, tkt)
        consume(*prev)
        pending.append(lambda o=out_ps, s=sums_ps, gg=g: epilogue(o, s, gg))
    while pending:
        pending.pop()()


def _oproj_chunk(nc, i, o_all_i, wo_sb, bo_sb, yT, ocp, ysp, opsps):
    """yT[:, i*TCH : (i+1)*TCH] = wo_local @ o_all_i (+ bo slice)."""
    o3 = o_all_i.rearrange("(o p) t -> p o t", p=128)
    for cc in range(TCH // PCH):
        oc_sb = ocp.tile([128, KT, PCH], CDT, tag="oc", name="oc")
        _load_x(nc, oc_sb, o3, cc * PCH, PCH)
        for e in range(HL):
            ps = opsps[e % 2].tile([128, TQ], F32, tag=f"sums{e % 2}",
                                   name=f"sums{e % 2}")[:, :PCH]
            for k in range(KT):
                nc.tensor.matmul(ps[:], wo_sb[:, k, e * 128:(e + 1) * 128],
                                 oc_sb[:, k, :], start=(k == 0), stop=(k == KT - 1))
            y_sb = ysp.tile([128, PCH], F32, tag="ys", name="ys")
            nc.vector.tensor_scalar_add(y_sb[:], ps[:], bo_sb[:, e:e + 1])
            t_lo = i * TCH + cc * PCH
            nc.gpsimd.dma_start(yT[e * 128:(e + 1) * 128, t_lo:t_lo + PCH], y_sb[:])


def _host_inputs(core, inputs):
    """Per-core numpy input map (all fp32; f32r is an interpretation)."""
    x = np.asarray(inputs["x"], np.float32)
    xT = np.ascontiguousarray(x.reshape(T, D).T)

    r0, r1 = core * EQ, (core + 1) * EQ
    kv0, kv1 = core * EKV, (core + 1) * EKV
    wqT = np.ascontiguousarray(np.asarray(inputs["wq"], np.float32)[r0:r1].T)
    wkT = np.ascontiguousarray(np.asarray(inputs["wk"], np.float32)[kv0:kv1].T)
    wvT = np.ascontiguousarray(np.asarray(inputs["wv"], np.float32)[kv0:kv1].T)
    woT = np.ascontiguousarray(np.asarray(inputs["wo"], np.float32)[r0:r1].T)
    bq = np.ascontiguousarray(np.asarray(inputs["bq"], np.float32)[r0:r1].reshape(HL, 128).T)
    bk = np.ascontiguousarray(np.asarray(inputs["bk"], np.float32)[kv0:kv1].reshape(1, 128).T)
    bv = np.ascontiguousarray(np.asarray(inputs["bv"], np.float32)[kv0:kv1].reshape(1, 128).T)
    bo = np.ascontiguousarray(np.asarray(inputs["bo"], np.float32)[r0:r1].reshape(HL, 128).T)

    i = np.arange(128)[:, None]
    j = np.arange(TQ)[None, :]
    masks = np.stack([(j < i + 128 * o) * np.float32(NEG) for o in range(4)], axis=1)
    import ml_dtypes
    masks = np.ascontiguousarray(masks.astype(ml_dtypes.bfloat16))

    return {
        "xT": xT, "wqT": wqT, "wkT": wkT, "wvT": wvT, "woT": woT,
        "bq": bq, "bk": bk, "bv": bv, "bo": bo,
        "masks": masks,
        "ones": np.ones((128, 128), np.float32),
        "onesf": np.ones((128, 128), np.float32),
        "ident": np.eye(128, dtype=np.float32),
    }


def kernel(**inputs) -> np.ndarray:
    if "nc" not in _cache:
        _cache["nc"] = build_bass()
    nc = _cache["nc"]
    in_maps = [_host_inputs(r, inputs) for r in range(NCORES)]
    last_err = None
    for attempt in range(3):
        try:
            res = run_bass_kernel_spmd(nc, in_maps, list(range(NCORES)))
            break
        except Exception as e:  # transient device wedges (NRT_EXEC_UNIT_UNRECOVERABLE)
            last_err = e
            if attempt == 2:
                raise
            try:  # the PJRT client dies with the wedge; re-establish it
                import time as _time
                import jax as _jax
                _jax.clear_caches()
                _jax.extend.backend.clear_backends()
                _time.sleep(5)
                _jax.devices()
            except Exception:
                pass
    yT = np.concatenate([res.results[r]["yT"] for r in range(NCORES)], axis=0)
    return np.ascontiguousarray(yT.T).reshape(B, S, D).astype(np.float32)


if __name__ == "__main__":
    import sys
    if len(sys.argv) > 1 and sys.argv[1] == "buildonly":
        import time
        t0 = time.perf_counter()
        nc = build_bass()
        print(f"build+bacc-compile ok in {time.perf_counter()-t0:.1f}s")
        sys.exit(0)
    import reference
    ins = {k: np.asarray(v) for k, v in reference.setup_inputs().items()}
    out = kernel(**ins)
    print("out", out.shape, out.dtype)
